# revision 2
# baseline (speedup 1.0000x reference)
"""Multi-scale patch pooling (gather + segment-mean) for CLIP-AD on 8 trn2 cores.

Reference, per batch element b:
    large[b, g, :] = mean over l of tokens[b, large_mask[l, g], :]   (9-elt mean, 169 groups)
    mid[b, g, :]   = mean over l of tokens[b, mid_mask[l, g], :]     (4-elt mean, 196 groups)
    cls[b, 0, :]   = mean over t of tokens[b, t, :]                  (225-elt mean)
    out = concat([large, mid, cls], axis=1)                          # [B, 366, D]

Per batch this is out_b = W @ tokens_b where W is a tiny [366, 225] membership
matrix built host-side from the masks (count/L entries — handles arbitrary /
duplicate indices; the 1/L mean scale is folded in). The whole pipeline runs in
bf16 (inputs cast host-side, output cast back on host): quantization costs
~2e-3 relative error against a 2e-2 budget, and halves HBM traffic — this
problem is memory-bound (68 MB/core at ~360 GB/s ≈ 190 us floor).

Device kernel (per core, 64 batches = 16 quads of 4):
  * Loads on the SP HWDGE ring (nc.sync), stores on the ACT HWDGE ring
    (nc.scalar) — two independent hardware DMA queues, so loads and stores
    overlap each other and every transfer spreads over all 16 SDMA engines.
  * Tokens packed host-side as tokq[q, p, (ki, b, d)]: contraction dim T=225
    split 113+112 across partitions, 4 batches side by side -> one contiguous
    1.62 MB load per quad.
  * Matmul on PE in bf16 (full-rate): for each m-tile mi (groups 3p+mi via a
    host-side column permute of W) and batch, accumulate the two k-chunks into
    a [122, 896] PSUM tile (2 banks; each matmul N tile 512/384 sits within
    one bank). 48 matmuls per quad, back-to-back so HAM stays warm.
  * PSUM evacuation alternates DVE / ACT, casting f32 -> bf16 into a [122,
    4*3*896] o tile; one contiguous 2.62 MB store per quad. Host untangles
    the (quad, partition, batch, mi) layout and casts back to f32.

Sharding: pure data parallel on batch — 64 batches per core; W replicated.
"""

import numpy as np

B, T, D = 512, 225, 896
GL, LL = 169, 9
GM, LM = 196, 4
G = GL + GM + 1  # 366
N_CORES = 8
BP = B // N_CORES  # 64
QB = 4             # batches per quad
NQ = BP // QB      # 16 quads per core

KP = 113                      # k-chunk partition count (225 -> 113 + 112)
MP = G // 3                   # 122 partitions per m-tile (groups strided by 3)
_K_TILES = ((0, 113), (113, 112))
_N_TILES = ((0, 512), (512, 384))
ROWE = 2 * QB * D             # packed row elems per partition (7168)
OCOL = QB * 3 * D             # o-tile cols (10752)

_CACHE = {}


def _get_nc():
    if "nc" in _CACHE:
        return _CACHE["nc"]
    from contextlib import ExitStack

    import concourse.bacc as bacc
    import concourse.mybir as mybir
    import concourse.tile as tile

    f32 = mybir.dt.float32
    bf16 = mybir.dt.bfloat16

    nc = bacc.Bacc("TRN2", target_bir_lowering=False, debug=False)
    # tokq[q, p, ki*4D + b*D + d] = bf16 token (4q+b, ki*113+p, d)
    tokq = nc.dram_tensor("tokq", [NQ, KP, ROWE], bf16, kind="ExternalInput").ap()
    # w01T[t, mi*122 + p] = (count/L) of group 3p+mi at token t
    w01T = nc.dram_tensor("w01T", [T, G], bf16, kind="ExternalInput").ap()
    out = nc.dram_tensor("out", [NQ, MP, OCOL], bf16, kind="ExternalOutput").ap()

    NTOK = 5  # token quad slots
    NOB = 4   # o-tile slots

    with tile.TileContext(nc) as tc:
        with ExitStack() as ctx:
            tokp = ctx.enter_context(tc.tile_pool(name="tok", bufs=NTOK))
            obp = ctx.enter_context(tc.tile_pool(name="ob", bufs=NOB))
            wp = ctx.enter_context(tc.tile_pool(name="w", bufs=1))
            psp = ctx.enter_context(tc.tile_pool(name="ps", bufs=4, space="PSUM"))

            # Warm-up ops: first ACT/DVE instructions pick up table-load waits
            # in lowering; give them dummies with no cross-engine deps.
            warm = wp.tile([128, 2], f32, tag="warm")
            nc.gpsimd.memset(warm[:], 0.0)
            nc.scalar.activation(
                warm[:], warm[:], mybir.ActivationFunctionType.Copy
            )
            nc.vector.tensor_copy(warm[:], warm[:])

            w_sb = []
            for ki, (k0, ksz) in enumerate(_K_TILES):
                wt = wp.tile([128, G], bf16, tag=f"w{ki}")
                nc.sync.dma_start(wt[:ksz, :], w01T[k0 : k0 + ksz, :])
                w_sb.append(wt)

            LOOK = NTOK - 1
            tks = {}

            def emit_load(q):
                tk = tokp.tile([128, ROWE], bf16, name="tok", tag="tok")
                nc.sync.dma_start(tk[:KP, :], tokq[q])
                tks[q] = tk

            for q in range(LOOK):
                emit_load(q)

            cp = 0
            for q in range(NQ):
                if q + LOOK < NQ:
                    emit_load(q + LOOK)
                tk = tks.pop(q)
                o = obp.tile([128, OCOL], bf16, name="ob", tag="ob")
                for mi in range(3):
                    for h in range(2):
                        pss = [
                            psp.tile([128, 896], f32, name="ps", tag="ps")
                            for _ in range(2)
                        ]
                        for ki, (k0, ksz) in enumerate(_K_TILES):
                            for bi in range(2):
                                base = ki * QB * D + (2 * h + bi) * D
                                for n0, nsz in _N_TILES:
                                    nc.tensor.matmul(
                                        pss[bi][:MP, n0 : n0 + nsz],
                                        w_sb[ki][:ksz, mi * MP : (mi + 1) * MP],
                                        tk[:ksz, base + n0 : base + n0 + nsz],
                                        start=(ki == 0),
                                        stop=(ki == 1),
                                    )
                        for bi in range(2):
                            dst = o[
                                :MP,
                                ((2 * h + bi) * 3 + mi) * D : ((2 * h + bi) * 3 + mi + 1) * D,
                            ]
                            if cp % 2 == 0:
                                nc.vector.tensor_copy(dst, pss[bi][:MP, :])
                            else:
                                nc.scalar.activation(
                                    dst,
                                    pss[bi][:MP, :],
                                    mybir.ActivationFunctionType.Copy,
                                )
                            cp += 1
                nc.scalar.dma_start(out[q], o[:MP, :])

    nc.compile()
    _CACHE["nc"] = nc
    return nc


def _host_prep(tokens_full, large_mask, mid_mask):
    """Cast to bf16, pack tokens for quad loads, build weight matrix."""
    import ml_dtypes

    bf16 = ml_dtypes.bfloat16
    bsz = tokens_full.shape[0]
    tok_bf = np.asarray(tokens_full, np.float32).astype(bf16)

    # tokq[q, p, ki, b, d] = tok(4q+b, ki*113+p, d); k-chunk 1 row 112 is the
    # last valid row (225 = 113 + 112), partition 112 of chunk 1 zero-padded.
    t4 = tok_bf.reshape(bsz // QB, QB, T, D)
    tokq = np.zeros((bsz // QB, KP, 2, QB, D), bf16)
    tokq[:, :, 0] = t4[:, :, 0:KP].transpose(0, 2, 1, 3)
    tokq[:, :112, 1] = t4[:, :, KP:T].transpose(0, 2, 1, 3)
    tokq = tokq.reshape(bsz // QB, KP, ROWE)

    W = np.zeros((G, T), np.float64)
    rows = np.arange(GL)
    for l in range(large_mask.shape[0]):
        np.add.at(W, (rows, large_mask[l]), 1.0 / LL)
    rows = GL + np.arange(GM)
    for l in range(mid_mask.shape[0]):
        np.add.at(W, (rows, mid_mask[l]), 1.0 / LM)
    W[G - 1, :] = 1.0 / T

    # Permute groups so m-tile mi, partition p <-> group 3p+mi.
    perm = np.concatenate([np.arange(mi, G, 3) for mi in range(3)])
    w01T = np.ascontiguousarray(W[perm].T).astype(bf16)  # [T, G]
    return tokq, w01T


def _in_maps(tokq, w01T, n_cores=N_CORES):
    qp = tokq.shape[0] // n_cores
    return [
        {
            "tokq": np.ascontiguousarray(tokq[c * qp : (c + 1) * qp]),
            "w01T": w01T,
        }
        for c in range(n_cores)
    ]


def _unpack_out(res_out):
    """[NQ, 122, QB*3*D] bf16 device layout -> [BP, G, D] f32."""
    arr = np.asarray(res_out).reshape(NQ, MP, QB, 3, D)
    return (
        arr.transpose(0, 2, 1, 3, 4)
        .reshape(BP, G, D)
        .astype(np.float32)
    )


def kernel(**inputs):
    from concourse import bass_utils

    tokens_full = np.ascontiguousarray(np.asarray(inputs["patch_tokens"], np.float32))
    large = np.asarray(inputs["large_mask"]).astype(np.int64)
    mid = np.asarray(inputs["mid_mask"]).astype(np.int64)
    tokq, w01T = _host_prep(tokens_full, large, mid)

    nc = _get_nc()
    res = bass_utils.run_bass_kernel_spmd(
        nc, _in_maps(tokq, w01T), core_ids=list(range(N_CORES))
    )
    return np.concatenate(
        [_unpack_out(res.results[c]["out"]) for c in range(N_CORES)], axis=0
    )


# revision 3
# speedup vs baseline: 3.9114x; 3.9114x over previous
"""Multi-scale patch pooling (gather + segment-mean) for CLIP-AD on 8 trn2 cores.

Reference, per batch element b:
    large[b, g, :] = mean over l of tokens[b, large_mask[l, g], :]   (9-elt mean, 169 groups)
    mid[b, g, :]   = mean over l of tokens[b, mid_mask[l, g], :]     (4-elt mean, 196 groups)
    cls[b, 0, :]   = mean over t of tokens[b, t, :]                  (225-elt mean)
    out = concat([large, mid, cls], axis=1)                          # [B, 366, D]

Per batch this is out_b = W @ tokens_b where W is a tiny [366, 225] membership
matrix built host-side from the masks (count/L entries — handles arbitrary /
duplicate indices; the 1/L mean scale is folded in). The whole pipeline runs in
bf16 (inputs cast host-side, output cast back on host): quantization costs
~2.6e-3 relative error against a 2e-2 budget, and halves HBM traffic — this
problem is memory-bound (68 MB/core at ~360 GB/s ≈ 190 us floor).

Device kernel (per core, 64 batches = 16 quads of 4):
  * All bulk DMA on gpsimd/SWDGE: software descriptor generation spreads each
    transfer's descriptors over all 16 SDMA engines (HWDGE's PDMA2D path pins
    a whole transfer to ONE engine at ~27 GB/s — measured, avoid).
  * Tokens packed host-side as tokq[q, p, (ki, b, d)]: contraction dim T=225
    split 113+112 across partitions, 4 batches side by side -> one 1.62 MB
    load per quad (113 descriptors of 14.4 KB). 64 B row pad keeps source
    runs non-contiguous so the AP normalizer can't merge them.
  * Stores are emitted one quad late so their sem-waits are satisfied on
    arrival — the gpsimd queue is in-order and a waiting store would block
    every load queued behind it.
  * Matmul on PE in bf16 (full-rate): for each m-tile mi (groups 3p+mi via a
    host-side column permute of W) and batch, accumulate the two k-chunks into
    a [122, 896] PSUM tile (2 banks; each matmul N tile 512/384 sits within
    one bank). 48 back-to-back matmuls per quad keep HAM warm.
  * PSUM evacuation alternates DVE / ACT, casting f32 -> bf16 into a [122,
    4*3*896] o tile; one contiguous 2.62 MB store per quad. Host untangles
    the (quad, partition, batch, mi) layout and casts back to f32.

Sharding: pure data parallel on batch — 64 batches per core; W replicated.
"""

import numpy as np

B, T, D = 512, 225, 896
GL, LL = 169, 9
GM, LM = 196, 4
G = GL + GM + 1  # 366
N_CORES = 8
BP = B // N_CORES  # 64
QB = 4             # batches per quad
NQ = BP // QB      # 16 quads per core

KP = 113                      # k-chunk partition count (225 -> 113 + 112)
MP = G // 3                   # 122 partitions per m-tile (groups strided by 3)
_K_TILES = ((0, 113), (113, 112))
_N_TILES = ((0, 512), (512, 384))
ROWE = 2 * QB * D             # packed row elems per partition (7168)
ROWPAD = 32                   # bf16 elems of pad per packed row (64 B)
OCOL = QB * 3 * D             # o-tile cols (10752)

_CACHE = {}


def _get_nc():
    if "nc" in _CACHE:
        return _CACHE["nc"]
    from contextlib import ExitStack

    import concourse.bacc as bacc
    import concourse.mybir as mybir
    import concourse.tile as tile

    f32 = mybir.dt.float32
    bf16 = mybir.dt.bfloat16

    nc = bacc.Bacc("TRN2", target_bir_lowering=False, debug=False)
    # tokq[q, p, ki*4D + b*D + d] = bf16 token (4q+b, ki*113+p, d)
    tokq = nc.dram_tensor(
        "tokq", [NQ, KP, ROWE + ROWPAD], bf16, kind="ExternalInput"
    ).ap()
    # w01T[t, mi*122 + p] = (count/L) of group 3p+mi at token t
    w01T = nc.dram_tensor("w01T", [T, G], bf16, kind="ExternalInput").ap()
    out = nc.dram_tensor("out", [NQ, MP, OCOL], bf16, kind="ExternalOutput").ap()

    NTOK = 7  # token quad slots (one in-flight load each)
    NOB = 4   # o-tile slots

    with tile.TileContext(nc) as tc:
        with ExitStack() as ctx:
            tok_pools = [
                ctx.enter_context(tc.tile_pool(name=f"tokp{s}", bufs=1))
                for s in range(NTOK)
            ]
            obp = ctx.enter_context(tc.tile_pool(name="ob", bufs=NOB))
            wp = ctx.enter_context(tc.tile_pool(name="w", bufs=1))
            psp = ctx.enter_context(tc.tile_pool(name="ps", bufs=4, space="PSUM"))

            # Warm-up ops: first ACT/DVE instructions pick up table-load waits
            # in lowering; give them dummies with no cross-engine deps.
            warm = wp.tile([128, 2], f32, tag="warm")
            nc.gpsimd.memset(warm[:], 0.0)
            nc.scalar.activation(
                warm[:], warm[:], mybir.ActivationFunctionType.Copy
            )
            nc.vector.tensor_copy(warm[:], warm[:])

            w_sb = []
            for ki, (k0, ksz) in enumerate(_K_TILES):
                wt = wp.tile([128, G], bf16, tag=f"w{ki}")
                nc.gpsimd.dma_start(wt[:ksz, :], w01T[k0 : k0 + ksz, :])
                w_sb.append(wt)

            LOOK = 4
            tks = {}

            def emit_load(q):
                tk = tok_pools[q % NTOK].tile(
                    [128, ROWE], bf16, name="tok", tag="tok"
                )
                nc.gpsimd.dma_start(tk[:KP, :], tokq[q, :, :ROWE])
                tks[q] = tk

            pending_stores = []

            def flush_stores():
                for dst, src in pending_stores:
                    nc.gpsimd.dma_start(dst, src)
                pending_stores.clear()

            for q in range(LOOK):
                emit_load(q)

            cp = 0
            for q in range(NQ):
                if q + LOOK < NQ:
                    emit_load(q + LOOK)
                flush_stores()
                tk = tks.pop(q)
                o = obp.tile([128, OCOL], bf16, name="ob", tag="ob")
                for mi in range(3):
                    for h in range(2):
                        pss = [
                            psp.tile([128, 896], f32, name="ps", tag="ps")
                            for _ in range(2)
                        ]
                        for ki, (k0, ksz) in enumerate(_K_TILES):
                            for bi in range(2):
                                base = ki * QB * D + (2 * h + bi) * D
                                for n0, nsz in _N_TILES:
                                    nc.tensor.matmul(
                                        pss[bi][:MP, n0 : n0 + nsz],
                                        w_sb[ki][:ksz, mi * MP : (mi + 1) * MP],
                                        tk[:ksz, base + n0 : base + n0 + nsz],
                                        start=(ki == 0),
                                        stop=(ki == 1),
                                    )
                        for bi in range(2):
                            dst = o[
                                :MP,
                                ((2 * h + bi) * 3 + mi) * D : ((2 * h + bi) * 3 + mi + 1)
                                * D,
                            ]
                            if cp % 2 == 0:
                                nc.vector.tensor_copy(dst, pss[bi][:MP, :])
                            else:
                                nc.scalar.activation(
                                    dst,
                                    pss[bi][:MP, :],
                                    mybir.ActivationFunctionType.Copy,
                                )
                            cp += 1
                pending_stores.append((out[q], o[:MP, :]))
            flush_stores()

    nc.compile()
    _CACHE["nc"] = nc
    return nc


def _host_prep(tokens_full, large_mask, mid_mask):
    """Cast to bf16, pack tokens for quad loads, build weight matrix."""
    import ml_dtypes

    bf16 = ml_dtypes.bfloat16
    bsz = tokens_full.shape[0]
    tok_bf = np.asarray(tokens_full, np.float32).astype(bf16)

    # tokq[q, p, ki, b, d] = tok(4q+b, ki*113+p, d); k-chunk 1 row 112 is the
    # last valid row (225 = 113 + 112), partition 112 of chunk 1 zero-padded.
    t4 = tok_bf.reshape(bsz // QB, QB, T, D)
    tokq = np.zeros((bsz // QB, KP, ROWE + ROWPAD), bf16)
    tq = tokq[:, :, :ROWE].reshape(bsz // QB, KP, 2, QB, D)
    tq[:, :, 0] = t4[:, :, 0:KP].transpose(0, 2, 1, 3)
    tq[:, :112, 1] = t4[:, :, KP:T].transpose(0, 2, 1, 3)

    W = np.zeros((G, T), np.float64)
    rows = np.arange(GL)
    for l in range(large_mask.shape[0]):
        np.add.at(W, (rows, large_mask[l]), 1.0 / LL)
    rows = GL + np.arange(GM)
    for l in range(mid_mask.shape[0]):
        np.add.at(W, (rows, mid_mask[l]), 1.0 / LM)
    W[G - 1, :] = 1.0 / T

    # Permute groups so m-tile mi, partition p <-> group 3p+mi.
    perm = np.concatenate([np.arange(mi, G, 3) for mi in range(3)])
    w01T = np.ascontiguousarray(W[perm].T).astype(bf16)  # [T, G]
    return tokq, w01T


def _in_maps(tokq, w01T, n_cores=N_CORES):
    qp = tokq.shape[0] // n_cores
    return [
        {
            "tokq": np.ascontiguousarray(tokq[c * qp : (c + 1) * qp]),
            "w01T": w01T,
        }
        for c in range(n_cores)
    ]


def _unpack_out(res_out):
    """[NQ, 122, QB*3*D] bf16 device layout -> [BP, G, D] f32."""
    arr = np.asarray(res_out).reshape(NQ, MP, QB, 3, D)
    return (
        arr.transpose(0, 2, 1, 3, 4)
        .reshape(BP, G, D)
        .astype(np.float32)
    )


def kernel(**inputs):
    from concourse import bass_utils

    tokens_full = np.ascontiguousarray(np.asarray(inputs["patch_tokens"], np.float32))
    large = np.asarray(inputs["large_mask"]).astype(np.int64)
    mid = np.asarray(inputs["mid_mask"]).astype(np.int64)
    tokq, w01T = _host_prep(tokens_full, large, mid)

    nc = _get_nc()
    res = bass_utils.run_bass_kernel_spmd(
        nc, _in_maps(tokq, w01T), core_ids=list(range(N_CORES))
    )
    return np.concatenate(
        [_unpack_out(res.results[c]["out"]) for c in range(N_CORES)], axis=0
    )
